# revision 10
# baseline (speedup 1.0000x reference)
"""Cost-volume kernel for Trainium2 (Bass/Tile), 8-core SPMD.

Problem: left/right features [B=2, C=32, H=128, W=256] f32.
Output [B, 2C=64, D=48, H, W] where for disparity d in [-8, 40):
  out[:, 0:C,  d+8, h, x] = left[:, :, h, x]   if 0 <= x-d < W else 0
  out[:, C:2C, d+8, h, x] = right[:, :, h, x-d] if 0 <= x-d < W else 0

Sharding: channels split 4-per-core (8 cores, identical program).
Each core builds the full disparity band for its 4 left + 4 right
channels.

Pure data-movement kernel, HBM-write-bound. The rel-err tolerance
(2e-2) admits fp16 (rel err ~5e-4), so the whole device pipeline runs
in fp16: host casts inputs, device moves 2-byte elements, host upcasts
the gathered result. That halves the obligatory HBM write traffic
(96 -> 48 MiB/core; per-NC HBM cap ~358 GB/s -> ~141us write floor).

Structure (evolved from a 318us f32 / 170us first-f16 kernel via NTFF
trace analysis):
  - All output stores run on the two HWDGE rings (right stream on SP,
    left stream on ACT), in parallel and with no cross-stream
    head-of-line blocking. SWDGE (gpsimd) only carries the three
    input loads: SDMA engines 7/15 are ~10-20% slower under
    SWDGE-heavy traffic (descriptor-ring AXI port contention), which
    in an earlier revision left engine 15 draining a ~0.5 MiB backlog
    alone for ~15us after the other engines finished.
  - Two dependency-free DRAM->DRAM prologues on the SP ring store
    both d=0 slices (the left d=0 slice is the raw left image - its
    mask is all-valid): 1 MiB of obligatory output writes that fill
    the SDMA engines during the load/copy warm-up.
  - Stores are batched 4 disparity slices per dma_start. Outputs are
    disparity-major in DRAM ([D,B,C',H,W]) so the batched AP merges
    to 3 dims while descriptors stay 4 KiB-contiguous. Host
    transposes the d-major result when gathering.
  - Right slices are DVE-staged from host-padded right images into
    4-slot staging tiles (zero margins come free from the padding).
    The host supplies TWO padded copies offset by one column, each
    contiguous in DRAM (fast large-descriptor loads), so every
    shifted window starts 4-byte-aligned regardless of disparity
    parity -- this keeps the DVE copies in 4x mode (2B dtype, step 1,
    4B align, ~0.6us per slice).
  - Positive-d left quads live in 3 rotating work tiles initialized
    from a pristine left tile; slot reuse zeroes 12 fresh columns on
    ACT. Negative-d left quads are built in the staging pool.
"""

import numpy as np

B, C, H, W = 2, 32, 128, 256
MIN_D, MAX_D = -8, 40
D = MAX_D - MIN_D  # 48
N_CORES = 8
CPC = C // N_CORES  # 4 channels of each image per core
BC = B * CPC  # 8 (b, c) pairs per core

PAD_E = 40  # even-d window start: a = PAD_E - d, even and in [2, 48]
PAD_O = 39  # odd-d window start: a = PAD_O - d, even and in [0, 46]
WT = 304    # padded row width (608 B rows, 4B-aligned)

HL = 8            # h rows held per partition
HH = H // HL      # 16
NPART = BC * HH   # 128 partitions: p = (b*CPC + c)*HH + h_hi

STAGE_BUFS = 12  # staging rotation depth (each buf holds 2 d)

_CACHE = {}


def _build_nc():
    import concourse.bacc as bacc
    import concourse.tile as tile
    import concourse.mybir as mybir

    f16 = mybir.dt.float16
    nc = bacc.Bacc(
        "TRN2",
        target_bir_lowering=False,
        debug=False,
        enable_asserts=False,
        num_devices=N_CORES,
    )
    left_in = nc.dram_tensor("left_in", [B, CPC, H, W], f16, kind="ExternalInput")
    # host-padded right images, width WT=304: data at [PAD_E, PAD_E+W)
    # resp. [PAD_O, PAD_O+W), zeros outside. Two copies offset by one
    # column give every disparity parity a 4B-aligned shifted window.
    right_e = nc.dram_tensor("right_e", [B, CPC, H, WT], f16, kind="ExternalInput")
    right_o = nc.dram_tensor("right_o", [B, CPC, H, WT], f16, kind="ExternalInput")
    # raw right, only for the DRAM->DRAM prologue store of di=8 (d=0)
    right_raw = nc.dram_tensor("right_raw", [B, CPC, H, W], f16, kind="ExternalInput")
    # disparity-major outputs; host transposes back when gathering
    left_out = nc.dram_tensor(
        "left_out", [D, B, CPC, H, W], f16, kind="ExternalOutput"
    )
    right_out = nc.dram_tensor(
        "right_out", [D, B, CPC, H, W], f16, kind="ExternalOutput"
    )

    def dest(out_t, di0, g):
        # dest AP for g consecutive disparity slices starting at di0,
        # iterated (b, c, h_hi, d, h_lo, w) to match the SBUF source
        # (partition = (b,c,h_hi), columns = (d_slot, h_lo, w)).
        ap = out_t.ap()[di0 : di0 + g, :, :, :, :]
        return ap.rearrange("g b c (hh hl) w -> b c hh g hl w", hl=HL)

    with tile.TileContext(nc) as tc:
        with (
            tc.tile_pool(name="pool", bufs=1) as pool,
            tc.tile_pool(name="stpool", bufs=STAGE_BUFS) as stpool,
        ):
            # ---- DRAM->DRAM prologues: dependency-free output slices
            # that fill the SDMA engines during the load/copy warm-up
            # (the first ~14us of the program used to run far below the
            # HBM rate). Both d=0 slices are raw input images; d=-8/-7
            # are shifted windows of the padded right images in DRAM
            # (strided 512B descriptors - fine for ramp filling).
            # Right-stream prologues go on the SP ring ahead of the
            # staged right batches, the left one on the ACT ring ahead
            # of the left quads. Capped descriptors keep the queue
            # round-robin fair against the input loads.
            nc.sync.dma_start(
                right_out.ap()[0 - MIN_D], right_raw.ap(), max_dma_last_dim=2048
            )
            nc.scalar.dma_start(
                left_out.ap()[0 - MIN_D], left_in.ap(), max_dma_last_dim=2048
            )
            nc.sync.dma_start(
                right_out.ap()[0], right_e.ap()[:, :, :, 48 : 48 + W]
            )
            nc.sync.dma_start(
                right_out.ap()[1], right_o.ap()[:, :, :, 46 : 46 + W]
            )

            # ---- tiles ----
            re_t = pool.tile([NPART, HL * WT], f16, tag="re")
            ro_t = pool.tile([NPART, HL * WT], f16, tag="ro")
            re3 = re_t[:].rearrange("p (h w) -> p h w", h=HL)
            ro3 = ro_t[:].rearrange("p (h w) -> p h w", h=HL)
            # pristine left image (never zeroed; source for slot inits)
            lt = pool.tile([NPART, HL * W], f16, tag="lt")
            lt3 = lt[:].rearrange("p (h w) -> p h w", h=HL)
            zt = pool.tile([NPART, HL * 16], f16, tag="zt")
            zt3 = zt[:].rearrange("p (h w) -> p h w", h=HL)

            # positive-d left work tiles: 3 rotating quads of 4 d-slots
            lp = [
                pool.tile([NPART, 4 * HL * W], f16, tag=f"lp{j}", name=f"lp{j}")
                for j in range(3)
            ]
            lp4 = [t[:].rearrange("p (g h w) -> p g h w", g=4, h=HL) for t in lp]

            # ---- loads (SWDGE; all contiguous in DRAM) ----
            nc.gpsimd.dma_start(re_t[:], right_e.ap())
            nc.gpsimd.dma_start(ro_t[:], right_o.ap())
            nc.gpsimd.dma_start(lt[:], left_in.ap())
            nc.vector.memset(zt[:], 0.0)

            def zero_cols(t4, g, a, b):
                if a < b:
                    nc.scalar.copy(t4[:, g, :, a:b], zt3[:, :, 0 : b - a])

            seen = set()

            # store units are ~1 MiB (2 disparity slices) and rotate
            # round-robin over the three DMA queues, weighted toward
            # the SWDGE queue ([gpsimd, sync, gpsimd, scalar]): coarse
            # per-stream queue assignment left the queues oscillating
            # out of phase (one queue's emission blocks ~16us on a
            # full ring while the others surge, then starve).
            qcycle = [nc.gpsimd, nc.sync, nc.gpsimd, nc.scalar]
            qi = [0]

            def next_q():
                eng = qcycle[qi[0] % len(qcycle)]
                qi[0] += 1
                return eng

            def emit_left_half(q, h):
                # slots g=2h,2h+1 of quad q: d = 4q+1+2h+g'; quad tile
                # q%3, slots reused every 3 quads (d jumps by 12)
                T = q % 3
                ng = 1 if (q, h) == (9, 1) else 2  # d=39 rides alone
                for g in range(2 * h, 2 * h + ng):
                    d = 4 * q + 1 + g
                    if (T, g) not in seen:
                        nc.vector.tensor_copy(lp4[T][:, g, :, :], lt3[:])
                        seen.add((T, g))
                        zero_cols(lp4[T], g, 0, d)
                    else:
                        zero_cols(lp4[T], g, d - 12, d)
                next_q().dma_start(
                    dest(left_out, 4 * q + 1 + 2 * h - MIN_D, ng),
                    lp[T][:, 2 * h * HL * W : (2 * h + ng) * HL * W],
                )

            def emit_left_neg(n):
                # covers d = -8+2n, -7+2n, built in the staging pool
                st = stpool.tile([NPART, 2 * HL * W], f16, tag="st")
                st4 = st[:].rearrange("p (g h w) -> p g h w", g=2, h=HL)
                for k in range(2):
                    d = -8 + 2 * n + k
                    nc.vector.tensor_copy(st4[:, k, :, :], lt3[:])
                    zero_cols(st4, k, W + d, W)
                next_q().dma_start(dest(left_out, 2 * n, 2), st[:])

            def emit_right_batch(di0, g):
                st = stpool.tile([NPART, 2 * HL * W], f16, tag="st")
                st4 = st[:].rearrange("p (g h w) -> p g h w", g=2, h=HL)
                for k in range(g):
                    d = di0 + k + MIN_D
                    if d % 2 == 0:
                        src3, a = re3, PAD_E - d
                    else:
                        src3, a = ro3, PAD_O - d
                    nc.vector.tensor_copy(st4[:, k, :, :], src3[:, :, a : a + W])
                next_q().dma_start(
                    dest(right_out, di0, g),
                    st[:, 0 : g * HL * W],
                )

            # ---- emission schedule ----
            # di 8 (both tensors) and right di 0,1 come from the
            # prologues; the rest is interleaved 1:1 right/left in
            # 2-slice units so all three queues stay fed.
            rights = [(2, 2), (4, 2), (6, 2)] + [
                (di, 2) for di in range(9, 46, 2)
            ] + [(47, 1)]
            lefts = (
                [("q", q, h) for q in range(10) for h in (0, 1)]
                + [("n", n, None) for n in range(4)]
            )
            li = 0

            def emit_left_next():
                nonlocal li
                if li < len(lefts):
                    kind, a, b = lefts[li]
                    li += 1
                    if kind == "q":
                        emit_left_half(a, b)
                    else:
                        emit_left_neg(a)

            for ri in range(len(rights)):
                emit_right_batch(*rights[ri])
                emit_left_next()
            while li < len(lefts):
                emit_left_next()

    nc.compile()
    return nc


def _get_nc():
    if "nc" not in _CACHE:
        _CACHE["nc"] = _build_nc()
    return _CACHE["nc"]


def kernel(left_feat, right_feat):
    from concourse.bass_utils import run_bass_kernel_spmd

    left = np.asarray(left_feat)
    right = np.asarray(right_feat)
    assert left.shape == (B, C, H, W) and right.shape == (B, C, H, W)
    left16 = np.ascontiguousarray(left, dtype=np.float16)
    right16 = np.ascontiguousarray(right, dtype=np.float16)

    nc = _get_nc()
    right_e = np.zeros((B, C, H, WT), dtype=np.float16)
    right_e[:, :, :, PAD_E : PAD_E + W] = right16
    right_o = np.zeros((B, C, H, WT), dtype=np.float16)
    right_o[:, :, :, PAD_O : PAD_O + W] = right16
    in_maps = []
    for m in range(N_CORES):
        sl = slice(m * CPC, (m + 1) * CPC)
        in_maps.append(
            {
                "left_in": np.ascontiguousarray(left16[:, sl]),
                "right_e": np.ascontiguousarray(right_e[:, sl]),
                "right_o": np.ascontiguousarray(right_o[:, sl]),
                "right_raw": np.ascontiguousarray(right16[:, sl]),
            }
        )
    res = run_bass_kernel_spmd(nc, in_maps, core_ids=list(range(N_CORES))).results

    out = np.empty((B, 2 * C, D, H, W), dtype=np.float32)
    for m in range(N_CORES):
        sl = slice(m * CPC, (m + 1) * CPC)
        out[:, sl] = res[m]["left_out"].transpose(1, 2, 0, 3, 4)
        out[:, C + m * CPC : C + (m + 1) * CPC] = res[m]["right_out"].transpose(
            1, 2, 0, 3, 4
        )
    return out


# revision 12
# speedup vs baseline: 1.0509x; 1.0509x over previous
"""Cost-volume kernel for Trainium2 (Bass/Tile), 8-core SPMD.

Problem: left/right features [B=2, C=32, H=128, W=256] f32.
Output [B, 2C=64, D=48, H, W] where for disparity d in [-8, 40):
  out[:, 0:C,  d+8, h, x] = left[:, :, h, x]   if 0 <= x-d < W else 0
  out[:, C:2C, d+8, h, x] = right[:, :, h, x-d] if 0 <= x-d < W else 0

Sharding: channels split 4-per-core (8 cores, identical program).
Each core builds the full disparity band for its 4 left + 4 right
channels.

Pure data-movement kernel, HBM-write-bound. The rel-err tolerance
(2e-2) admits fp16 (rel err ~5e-4), so the whole device pipeline runs
in fp16: host casts inputs, device moves 2-byte elements, host upcasts
the gathered result. That halves the obligatory HBM write traffic
(96 -> 48 MiB/core; per-NC HBM cap ~358 GB/s -> ~141us write floor).

Structure (evolved from a 318us f32 / 170us first-f16 kernel via NTFF
trace analysis):
  - All output stores run on the two HWDGE rings (right stream on SP,
    left stream on ACT), in parallel and with no cross-stream
    head-of-line blocking. SWDGE (gpsimd) only carries the three
    input loads: SDMA engines 7/15 are ~10-20% slower under
    SWDGE-heavy traffic (descriptor-ring AXI port contention), which
    in an earlier revision left engine 15 draining a ~0.5 MiB backlog
    alone for ~15us after the other engines finished.
  - Two dependency-free DRAM->DRAM prologues on the SP ring store
    both d=0 slices (the left d=0 slice is the raw left image - its
    mask is all-valid): 1 MiB of obligatory output writes that fill
    the SDMA engines during the load/copy warm-up.
  - Stores are batched 4 disparity slices per dma_start. Outputs are
    disparity-major in DRAM ([D,B,C',H,W]) so the batched AP merges
    to 3 dims while descriptors stay 4 KiB-contiguous. Host
    transposes the d-major result when gathering.
  - Right slices are DVE-staged from host-padded right images into
    4-slot staging tiles (zero margins come free from the padding).
    The host supplies TWO padded copies offset by one column, each
    contiguous in DRAM (fast large-descriptor loads), so every
    shifted window starts 4-byte-aligned regardless of disparity
    parity -- this keeps the DVE copies in 4x mode (2B dtype, step 1,
    4B align, ~0.6us per slice).
  - Positive-d left quads live in 3 rotating work tiles initialized
    from a pristine left tile; slot reuse zeroes 12 fresh columns on
    ACT. Negative-d left quads are built in the staging pool.
"""

import numpy as np

B, C, H, W = 2, 32, 128, 256
MIN_D, MAX_D = -8, 40
D = MAX_D - MIN_D  # 48
N_CORES = 8
CPC = C // N_CORES  # 4 channels of each image per core
BC = B * CPC  # 8 (b, c) pairs per core

PAD_E = 40  # even-d window start: a = PAD_E - d, even and in [2, 48]
PAD_O = 39  # odd-d window start: a = PAD_O - d, even and in [0, 46]
WT = 304    # padded row width (608 B rows, 4B-aligned)

HL = 8            # h rows held per partition
HH = H // HL      # 16
NPART = BC * HH   # 128 partitions: p = (b*CPC + c)*HH + h_hi

STAGE_BUFS = 7  # staging rotation depth (each buf holds up to 4 d)

_CACHE = {}


def _build_nc():
    import concourse.bacc as bacc
    import concourse.tile as tile
    import concourse.mybir as mybir

    f16 = mybir.dt.float16
    nc = bacc.Bacc(
        "TRN2",
        target_bir_lowering=False,
        debug=False,
        enable_asserts=False,
        num_devices=N_CORES,
    )
    left_in = nc.dram_tensor("left_in", [B, CPC, H, W], f16, kind="ExternalInput")
    # host-padded right images, width WT=304: data at [PAD_E, PAD_E+W)
    # resp. [PAD_O, PAD_O+W), zeros outside. Two copies offset by one
    # column give every disparity parity a 4B-aligned shifted window.
    right_e = nc.dram_tensor("right_e", [B, CPC, H, WT], f16, kind="ExternalInput")
    right_o = nc.dram_tensor("right_o", [B, CPC, H, WT], f16, kind="ExternalInput")
    # raw right, only for the DRAM->DRAM prologue store of di=8 (d=0)
    right_raw = nc.dram_tensor("right_raw", [B, CPC, H, W], f16, kind="ExternalInput")
    # disparity-major outputs; host transposes back when gathering
    left_out = nc.dram_tensor(
        "left_out", [D, B, CPC, H, W], f16, kind="ExternalOutput"
    )
    right_out = nc.dram_tensor(
        "right_out", [D, B, CPC, H, W], f16, kind="ExternalOutput"
    )

    def dest(out_t, di0, g):
        # dest AP for g consecutive disparity slices starting at di0,
        # iterated (b, c, h_hi, d, h_lo, w) to match the SBUF source
        # (partition = (b,c,h_hi), columns = (d_slot, h_lo, w)).
        ap = out_t.ap()[di0 : di0 + g, :, :, :, :]
        return ap.rearrange("g b c (hh hl) w -> b c hh g hl w", hl=HL)

    with tile.TileContext(nc) as tc:
        with (
            tc.tile_pool(name="pool", bufs=1) as pool,
            tc.tile_pool(name="stpool", bufs=STAGE_BUFS) as stpool,
        ):
            # ---- DRAM->DRAM prologues: dependency-free output slices
            # that fill the SDMA engines during the load/copy warm-up
            # (the first ~14us of the program used to run far below the
            # HBM rate). Both d=0 slices are raw input images; d=-8/-7
            # are shifted windows of the padded right images in DRAM
            # (strided 512B descriptors - fine for ramp filling).
            # Right-stream prologues go on the SP ring ahead of the
            # staged right batches, the left one on the ACT ring ahead
            # of the left quads. Capped descriptors keep the queue
            # round-robin fair against the input loads.
            nc.sync.dma_start(
                right_out.ap()[0 - MIN_D], right_raw.ap(), max_dma_last_dim=2048
            )
            nc.scalar.dma_start(
                left_out.ap()[0 - MIN_D], left_in.ap(), max_dma_last_dim=2048
            )
            nc.sync.dma_start(
                right_out.ap()[0], right_e.ap()[:, :, :, 48 : 48 + W]
            )
            nc.sync.dma_start(
                right_out.ap()[1], right_o.ap()[:, :, :, 46 : 46 + W]
            )

            # ---- tiles ----
            re_t = pool.tile([NPART, HL * WT], f16, tag="re")
            ro_t = pool.tile([NPART, HL * WT], f16, tag="ro")
            re3 = re_t[:].rearrange("p (h w) -> p h w", h=HL)
            ro3 = ro_t[:].rearrange("p (h w) -> p h w", h=HL)
            # pristine left image (never zeroed; source for slot inits)
            lt = pool.tile([NPART, HL * W], f16, tag="lt")
            lt3 = lt[:].rearrange("p (h w) -> p h w", h=HL)
            zt = pool.tile([NPART, HL * 16], f16, tag="zt")
            zt3 = zt[:].rearrange("p (h w) -> p h w", h=HL)

            # positive-d left work tiles: 3 rotating quads of 4 d-slots
            lp = [
                pool.tile([NPART, 4 * HL * W], f16, tag=f"lp{j}", name=f"lp{j}")
                for j in range(3)
            ]
            lp4 = [t[:].rearrange("p (g h w) -> p g h w", g=4, h=HL) for t in lp]

            # ---- loads (SWDGE; all contiguous in DRAM) ----
            nc.gpsimd.dma_start(re_t[:], right_e.ap())
            nc.gpsimd.dma_start(ro_t[:], right_o.ap())
            nc.gpsimd.dma_start(lt[:], left_in.ap())
            nc.vector.memset(zt[:], 0.0)

            def zero_cols(t4, g, a, b):
                if a < b:
                    nc.scalar.copy(t4[:, g, :, a:b], zt3[:, :, 0 : b - a])

            seen = set()

            # ALL stores go on the single SWDGE queue: its Q7 emission
            # self-paces against the descriptor-ring drain, so the
            # store stream cannot oscillate out of phase the way
            # multi-queue splits did (one queue starving while another
            # surges). SWDGE alone sustains the full ~400 B/ns HBM
            # rate; the HWDGE rings only carry the prologue ramp.

            def emit_left_quad(q):
                # covers d = 4q+1 .. 4q+4 (3 slots for q=9: d 37..39);
                # tile q%3; slots reused every 3 quads (d jumps by 12)
                T = q % 3
                ng = 3 if q == 9 else 4
                for g in range(ng):
                    d = 4 * q + 1 + g
                    if (T, g) not in seen:
                        nc.vector.tensor_copy(lp4[T][:, g, :, :], lt3[:])
                        seen.add((T, g))
                        zero_cols(lp4[T], g, 0, d)
                    else:
                        zero_cols(lp4[T], g, d - 12, d)
                nc.gpsimd.dma_start(
                    dest(left_out, 4 * q + 1 - MIN_D, ng),
                    lp[T][:, 0 : ng * HL * W],
                )

            def emit_left_neg(n):
                # covers d = -8+4n .. -5+4n, built in the staging pool
                st = stpool.tile([NPART, 4 * HL * W], f16, tag="st")
                st4 = st[:].rearrange("p (g h w) -> p g h w", g=4, h=HL)
                for k in range(4):
                    d = -8 + 4 * n + k
                    nc.vector.tensor_copy(st4[:, k, :, :], lt3[:])
                    zero_cols(st4, k, W + d, W)
                nc.gpsimd.dma_start(dest(left_out, 4 * n, 4), st[:])

            def emit_right_batch(di0, g):
                st = stpool.tile([NPART, 4 * HL * W], f16, tag="st")
                st4 = st[:].rearrange("p (g h w) -> p g h w", g=4, h=HL)
                for k in range(g):
                    d = di0 + k + MIN_D
                    if d % 2 == 0:
                        src3, a = re3, PAD_E - d
                    else:
                        src3, a = ro3, PAD_O - d
                    nc.vector.tensor_copy(st4[:, k, :, :], src3[:, :, a : a + W])
                nc.gpsimd.dma_start(
                    dest(right_out, di0, g),
                    st[:, 0 : g * HL * W],
                )

            # ---- emission schedule ----
            # di 8 (both tensors) and right di 0,1 come from the
            # prologues. Right batches: di {2,6,9,13,...,41,45}; left
            # quads: d 1..39, negatives d -8..-1 via the staging pool.
            rights = [(2, 4), (6, 2), (9, 4), (13, 4), (17, 4), (21, 4),
                      (25, 4), (29, 4), (33, 4), (37, 4), (41, 4), (45, 3)]
            emit_right_batch(*rights[0])
            emit_left_quad(0)
            emit_right_batch(*rights[1])
            emit_left_quad(1)
            emit_left_neg(0)
            emit_right_batch(*rights[2])
            emit_left_quad(2)
            emit_right_batch(*rights[3])
            emit_left_quad(3)
            emit_left_neg(1)
            li = 4
            for ri in range(4, len(rights)):
                emit_right_batch(*rights[ri])
                if li <= 9:
                    emit_left_quad(li)
                    li += 1

    nc.compile()
    return nc


def _get_nc():
    if "nc" not in _CACHE:
        _CACHE["nc"] = _build_nc()
    return _CACHE["nc"]


def kernel(left_feat, right_feat):
    from concourse.bass_utils import run_bass_kernel_spmd

    left = np.asarray(left_feat)
    right = np.asarray(right_feat)
    assert left.shape == (B, C, H, W) and right.shape == (B, C, H, W)
    left16 = np.ascontiguousarray(left, dtype=np.float16)
    right16 = np.ascontiguousarray(right, dtype=np.float16)

    nc = _get_nc()
    right_e = np.zeros((B, C, H, WT), dtype=np.float16)
    right_e[:, :, :, PAD_E : PAD_E + W] = right16
    right_o = np.zeros((B, C, H, WT), dtype=np.float16)
    right_o[:, :, :, PAD_O : PAD_O + W] = right16
    in_maps = []
    for m in range(N_CORES):
        sl = slice(m * CPC, (m + 1) * CPC)
        in_maps.append(
            {
                "left_in": np.ascontiguousarray(left16[:, sl]),
                "right_e": np.ascontiguousarray(right_e[:, sl]),
                "right_o": np.ascontiguousarray(right_o[:, sl]),
                "right_raw": np.ascontiguousarray(right16[:, sl]),
            }
        )
    res = run_bass_kernel_spmd(nc, in_maps, core_ids=list(range(N_CORES))).results

    out = np.empty((B, 2 * C, D, H, W), dtype=np.float32)
    for m in range(N_CORES):
        sl = slice(m * CPC, (m + 1) * CPC)
        out[:, sl] = res[m]["left_out"].transpose(1, 2, 0, 3, 4)
        out[:, C + m * CPC : C + (m + 1) * CPC] = res[m]["right_out"].transpose(
            1, 2, 0, 3, 4
        )
    return out


# revision 13
# speedup vs baseline: 1.0658x; 1.0141x over previous
"""Cost-volume kernel for Trainium2 (Bass/Tile), 8-core SPMD.

Problem: left/right features [B=2, C=32, H=128, W=256] f32.
Output [B, 2C=64, D=48, H, W] where for disparity d in [-8, 40):
  out[:, 0:C,  d+8, h, x] = left[:, :, h, x]   if 0 <= x-d < W else 0
  out[:, C:2C, d+8, h, x] = right[:, :, h, x-d] if 0 <= x-d < W else 0

Sharding: channels split 4-per-core (8 cores, identical program).
Each core builds the full disparity band for its 4 left + 4 right
channels.

Pure data-movement kernel, HBM-write-bound. The rel-err tolerance
(2e-2) admits fp16 (rel err ~5e-4), so the whole device pipeline runs
in fp16: host casts inputs, device moves 2-byte elements, host upcasts
the gathered result. That halves the obligatory HBM write traffic
(96 -> 48 MiB/core; ~400 B/ns of aggregate SDMA throughput observed
-> ~130us store floor).

Structure (evolved from a 318us f32 / 170us first-f16 kernel via NTFF
trace analysis):
  - ALL stores go on the single SWDGE queue: its Q7 emission
    self-paces against the descriptor-ring drain, so the store stream
    cannot oscillate out of phase the way multi-queue splits did (one
    queue's emission blocking ~16us on a full ring while the others
    surge, then starve). SWDGE alone sustains the full ~400 B/ns.
  - The warm-up ramp is covered by ten direct-window single-slice
    stores (di 0..8 right, di 8 left) that read straight from the
    loaded image tiles with a shifted AP: no DVE staging, no extra
    HBM reads, ready as soon as each load's semaphore fires. Their
    512B descriptors are ~25% less engine-efficient, which is free
    during the otherwise idle ramp.
  - Remaining stores are batched 4 disparity slices per dma_start.
    Outputs are disparity-major in DRAM ([D,B,C',H,W]) so the batched
    AP merges to 3 dims while descriptors stay 4 KiB-contiguous. Host
    transposes the d-major result when gathering.
  - Right slices are DVE-staged from host-padded right images into
    4-slot staging tiles (zero margins come free from the padding).
    The host supplies TWO padded copies offset by one column, each
    contiguous in DRAM (fast large-descriptor loads), so every
    shifted window starts 4-byte-aligned regardless of disparity
    parity -- this keeps the DVE copies in 4x mode (2B dtype, step 1,
    4B align, ~0.7us per slice).
  - Positive-d left quads live in 3 rotating work tiles initialized
    from the pristine left tile; slot reuse zeroes 12 fresh columns
    on ACT. Negative-d left quads are built in the staging pool.
"""

import numpy as np

B, C, H, W = 2, 32, 128, 256
MIN_D, MAX_D = -8, 40
D = MAX_D - MIN_D  # 48
N_CORES = 8
CPC = C // N_CORES  # 4 channels of each image per core
BC = B * CPC  # 8 (b, c) pairs per core

PAD_E = 40  # even-d window start: a = PAD_E - d, even and in [2, 48]
PAD_O = 39  # odd-d window start: a = PAD_O - d, even and in [0, 46]
WT = 304    # padded row width (608 B rows, 4B-aligned)

HL = 8            # h rows held per partition
HH = H // HL      # 16
NPART = BC * HH   # 128 partitions: p = (b*CPC + c)*HH + h_hi

STAGE_BUFS = 7  # staging rotation depth (each buf holds up to 4 d)

_CACHE = {}


def _build_nc():
    import concourse.bacc as bacc
    import concourse.tile as tile
    import concourse.mybir as mybir

    f16 = mybir.dt.float16
    nc = bacc.Bacc(
        "TRN2",
        target_bir_lowering=False,
        debug=False,
        enable_asserts=False,
        num_devices=N_CORES,
    )
    left_in = nc.dram_tensor("left_in", [B, CPC, H, W], f16, kind="ExternalInput")
    # host-padded right images, width WT=304: data at [PAD_E, PAD_E+W)
    # resp. [PAD_O, PAD_O+W), zeros outside. Two copies offset by one
    # column give every disparity parity a 4B-aligned shifted window.
    right_e = nc.dram_tensor("right_e", [B, CPC, H, WT], f16, kind="ExternalInput")
    right_o = nc.dram_tensor("right_o", [B, CPC, H, WT], f16, kind="ExternalInput")
    # disparity-major outputs; host transposes back when gathering
    left_out = nc.dram_tensor(
        "left_out", [D, B, CPC, H, W], f16, kind="ExternalOutput"
    )
    right_out = nc.dram_tensor(
        "right_out", [D, B, CPC, H, W], f16, kind="ExternalOutput"
    )

    def dest(out_t, di0, g):
        # dest AP for g consecutive disparity slices starting at di0,
        # iterated (b, c, h_hi, d, h_lo, w) to match the SBUF source
        # (partition = (b,c,h_hi), columns = (d_slot, h_lo, w)).
        ap = out_t.ap()[di0 : di0 + g, :, :, :, :]
        return ap.rearrange("g b c (hh hl) w -> b c hh g hl w", hl=HL)

    with tile.TileContext(nc) as tc:
        with (
            tc.tile_pool(name="pool", bufs=1) as pool,
            tc.tile_pool(name="stpool", bufs=STAGE_BUFS) as stpool,
        ):
            # ---- tiles ----
            re_t = pool.tile([NPART, HL * WT], f16, tag="re")
            ro_t = pool.tile([NPART, HL * WT], f16, tag="ro")
            re3 = re_t[:].rearrange("p (h w) -> p h w", h=HL)
            ro3 = ro_t[:].rearrange("p (h w) -> p h w", h=HL)
            # pristine left image (never zeroed; d=0 slice + init src)
            lt = pool.tile([NPART, HL * W], f16, tag="lt")
            lt3 = lt[:].rearrange("p (h w) -> p h w", h=HL)
            zt = pool.tile([NPART, HL * 16], f16, tag="zt")
            zt3 = zt[:].rearrange("p (h w) -> p h w", h=HL)

            # positive-d left work tiles: 3 rotating quads of 4 d-slots
            lp = [
                pool.tile([NPART, 4 * HL * W], f16, tag=f"lp{j}", name=f"lp{j}")
                for j in range(3)
            ]
            lp4 = [t[:].rearrange("p (g h w) -> p g h w", g=4, h=HL) for t in lp]

            # ---- loads (all contiguous in DRAM) ----
            nc.gpsimd.dma_start(re_t[:], right_e.ap())
            nc.gpsimd.dma_start(lt[:], left_in.ap())
            nc.gpsimd.dma_start(ro_t[:], right_o.ap())
            nc.vector.memset(zt[:], 0.0)

            def window(d):
                src3 = re3 if d % 2 == 0 else ro3
                a = (PAD_E if d % 2 == 0 else PAD_O) - d
                return src3[:, :, a : a + W]

            def emit_right_direct(di):
                # single-slice store straight from the padded image
                # tile's shifted window (512B descriptors; ramp filler)
                nc.gpsimd.dma_start(dest(right_out, di, 1), window(di + MIN_D))

            def zero_cols(t4, g, a, b):
                if a < b:
                    nc.scalar.copy(t4[:, g, :, a:b], zt3[:, :, 0 : b - a])

            seen = set()

            def emit_left_quad(q):
                # covers d = 4q+1 .. 4q+4 (3 slots for q=9: d 37..39);
                # tile q%3; slots reused every 3 quads (d jumps by 12)
                T = q % 3
                ng = 3 if q == 9 else 4
                for g in range(ng):
                    d = 4 * q + 1 + g
                    if (T, g) not in seen:
                        nc.vector.tensor_copy(lp4[T][:, g, :, :], lt3[:])
                        seen.add((T, g))
                        zero_cols(lp4[T], g, 0, d)
                    else:
                        zero_cols(lp4[T], g, d - 12, d)
                nc.gpsimd.dma_start(
                    dest(left_out, 4 * q + 1 - MIN_D, ng),
                    lp[T][:, 0 : ng * HL * W],
                )

            def emit_left_neg(n):
                # covers d = -8+4n .. -5+4n, built in the staging pool
                st = stpool.tile([NPART, 4 * HL * W], f16, tag="st")
                st4 = st[:].rearrange("p (g h w) -> p g h w", g=4, h=HL)
                for k in range(4):
                    d = -8 + 4 * n + k
                    nc.vector.tensor_copy(st4[:, k, :, :], lt3[:])
                    zero_cols(st4, k, W + d, W)
                nc.gpsimd.dma_start(dest(left_out, 4 * n, 4), st[:])

            def emit_right_batch(di0, g):
                st = stpool.tile([NPART, 4 * HL * W], f16, tag="st")
                st4 = st[:].rearrange("p (g h w) -> p g h w", g=4, h=HL)
                for k in range(g):
                    nc.vector.tensor_copy(
                        st4[:, k, :, :], window(di0 + k + MIN_D)
                    )
                nc.gpsimd.dma_start(
                    dest(right_out, di0, g),
                    st[:, 0 : g * HL * W],
                )

            # ---- emission schedule ----
            # Ramp: direct-window singles for right di 0..8 (even from
            # re, odd from ro) and the left d=0 slice straight from
            # the pristine lt tile. Then staged 4-slice batches.
            for di in (0, 2, 4, 6, 8):
                emit_right_direct(di)
            nc.gpsimd.dma_start(dest(left_out, 8, 1), lt[:])
            for di in (1, 3, 5, 7):
                emit_right_direct(di)

            rights = [(9, 4), (13, 4), (17, 4), (21, 4), (25, 4),
                      (29, 4), (33, 4), (37, 4), (41, 4), (45, 3)]
            emit_right_batch(*rights[0])
            emit_left_quad(0)
            emit_right_batch(*rights[1])
            emit_left_quad(1)
            emit_left_neg(0)
            emit_right_batch(*rights[2])
            emit_left_quad(2)
            emit_right_batch(*rights[3])
            emit_left_quad(3)
            emit_left_neg(1)
            li = 4
            for ri in range(4, len(rights)):
                emit_right_batch(*rights[ri])
                if li <= 9:
                    emit_left_quad(li)
                    li += 1

    nc.compile()
    return nc


def _get_nc():
    if "nc" not in _CACHE:
        _CACHE["nc"] = _build_nc()
    return _CACHE["nc"]


def kernel(left_feat, right_feat):
    from concourse.bass_utils import run_bass_kernel_spmd

    left = np.asarray(left_feat)
    right = np.asarray(right_feat)
    assert left.shape == (B, C, H, W) and right.shape == (B, C, H, W)
    left16 = np.ascontiguousarray(left, dtype=np.float16)
    right16 = np.ascontiguousarray(right, dtype=np.float16)

    nc = _get_nc()
    right_e = np.zeros((B, C, H, WT), dtype=np.float16)
    right_e[:, :, :, PAD_E : PAD_E + W] = right16
    right_o = np.zeros((B, C, H, WT), dtype=np.float16)
    right_o[:, :, :, PAD_O : PAD_O + W] = right16
    in_maps = []
    for m in range(N_CORES):
        sl = slice(m * CPC, (m + 1) * CPC)
        in_maps.append(
            {
                "left_in": np.ascontiguousarray(left16[:, sl]),
                "right_e": np.ascontiguousarray(right_e[:, sl]),
                "right_o": np.ascontiguousarray(right_o[:, sl]),
            }
        )
    res = run_bass_kernel_spmd(nc, in_maps, core_ids=list(range(N_CORES))).results

    out = np.empty((B, 2 * C, D, H, W), dtype=np.float32)
    for m in range(N_CORES):
        sl = slice(m * CPC, (m + 1) * CPC)
        out[:, sl] = res[m]["left_out"].transpose(1, 2, 0, 3, 4)
        out[:, C + m * CPC : C + (m + 1) * CPC] = res[m]["right_out"].transpose(
            1, 2, 0, 3, 4
        )
    return out
